# revision 3
# baseline (speedup 1.0000x reference)
"""AttentiveTransformer (Linear -> ghost BatchNorm -> sparsemax) on 8 TRN2 cores.

Data-parallel over the batch: each core gets 2048 rows (16 ghost-BN chunks of
128 rows). Single matmul pass: y = (x - mean_chunk) @ W.T is computed once per
chunk; zp = y * prior is stored (fp16) while y's squares accumulate chunk
variances in PSUM via one-hot matmuls. Stats (invstd via Rsqrt) are computed
batched for all 16 chunks, then z = zp * invstd. The sparsemax threshold tau
(sum_j relu(z_j - tau) = 1) is found by Newton iteration from tau0 = THRESH,
exact for this piecewise-linear equation. Candidates (z > THRESH) are
compacted to `cap` slots (mask -> scan -> gpsimd local_scatter, slot 0 is a
shared trash slot for non-candidates) and iterations run on compacted values.

x/W/prior are downcast to fp16 on the host; x and W are transposed on the way
into SBUF by the DMA xbar (no PE transposes). Output is written fp16 and
upcast on the host.
"""
import numpy as np
from contextlib import ExitStack

import concourse.bass as bass
import concourse.bacc as bacc
import concourse.tile as tile
import concourse.mybir as mybir
import concourse.library_config as libcfg
from concourse.bass_utils import run_bass_kernel_spmd

N_CORES = 8
B, NA, F = 16384, 512, 2048
BL = B // N_CORES        # rows per core
VBS = 128                # ghost-BN virtual batch
KC = NA // 128           # k-chunks of 128
EPS = 1e-5

f32 = mybir.dt.float32
fp16 = mybir.dt.float16
i16 = mybir.dt.int16
ALU = mybir.AluOpType
ACTF = mybir.ActivationFunctionType


def build(nchunk=BL // VBS, n_iters=7, cap=72, group=4, thresh=1.45,
          half=1024, use_rsqrt=False):
    nc = bacc.Bacc("TRN2", target_bir_lowering=False)
    nhalf = F // half

    Bloc = nchunk * VBS
    x_d = nc.dram_tensor("x", [Bloc, NA], fp16, kind="ExternalInput")
    p_d = nc.dram_tensor("prior", [Bloc, F], fp16, kind="ExternalInput")
    w_d = nc.dram_tensor("w", [F, NA], fp16, kind="ExternalInput")
    o_d = nc.dram_tensor("out", [Bloc, F], fp16, kind="ExternalOutput")
    s16_d = nc.dram_tensor("s16scratch", [nchunk, F], fp16)

    with tile.TileContext(nc) as tc:
        with ExitStack() as ctx:
            ctx.enter_context(nc.allow_low_precision(
                reason="fp16 operands; validated against reference"))
            const = ctx.enter_context(tc.tile_pool(name="const", bufs=1))
            persist = ctx.enter_context(tc.tile_pool(name="persist", bufs=1))
            xp = ctx.enter_context(tc.tile_pool(name="xp", bufs=3))
            priorp = ctx.enter_context(tc.tile_pool(name="priorp", bufs=3))
            small = ctx.enter_context(tc.tile_pool(name="small", bufs=6))
            ysqp = ctx.enter_context(tc.tile_pool(name="ysqp", bufs=3))

            # one-hot columns: e_all[p, c, j] = (c == j)
            e_all = const.tile([128, nchunk, nchunk], fp16)
            nc.gpsimd.memset(e_all, 0.0)
            nc.gpsimd.affine_select(
                out=e_all, in_=e_all, compare_op=ALU.not_equal, fill=1.0,
                base=0, pattern=[[1, nchunk], [-1, nchunk]],
                channel_multiplier=0)

            eps_t = const.tile([nchunk, 1], f32)
            nc.vector.memset(eps_t, EPS)

            # ---- W load via DMA-transpose: wt[:, kc, f] = W[f, 128*kc+p] --
            wt = persist.tile([128, KC, F], fp16)
            for ft in range(F // 128):
                for kc in range(KC):
                    nc.sync.dma_start_transpose(
                        wt[:, kc, ft * 128:(ft + 1) * 128],
                        w_d[ft * 128:(ft + 1) * 128,
                            kc * 128:(kc + 1) * 128])

            # ---- phase A: per chunk: xT -> center -> matmul -> ysq/zp -----
            zp16 = persist.tile([128, nchunk, F], fp16)
            psvar_pool = tc.tile_pool(name="psvar", bufs=1, space="PSUM")
            psvar = psvar_pool.__enter__()
            pvar = psvar.tile([nchunk, nhalf, half], f32)
            with tc.tile_pool(name="psY", bufs=2, space="PSUM") as psY:
                for c in range(nchunk):
                    xt = xp.tile([128, KC, 128], fp16, tag="xt")
                    for kc in range(KC):
                        nc.sync.dma_start_transpose(
                            xt[:, kc, :],
                            x_d[c * VBS:(c + 1) * VBS,
                                kc * 128:(kc + 1) * 128])
                    prior_t = priorp.tile([128, F], fp16, tag="prior")
                    nc.sync.dma_start(prior_t, p_d[c * VBS:(c + 1) * VBS, :])
                    xsum = small.tile([128, KC], fp16, tag="xsum")
                    nc.vector.tensor_reduce(
                        out=xsum, in_=xt, axis=mybir.AxisListType.X,
                        op=ALU.add)
                    xbar = small.tile([128, KC], fp16, tag="xbar")
                    nc.vector.tensor_scalar(
                        out=xbar, in0=xsum, scalar1=1.0 / VBS, scalar2=None,
                        op0=ALU.mult)
                    xtc = xp.tile([128, KC, 128], fp16, tag="xtc")
                    xb = xbar[:, :]
                    xb_b = bass.AP(tensor=xb.tensor, offset=xb.offset,
                                   ap=list(xb.ap) + [[0, 128]])
                    nc.vector.scalar_tensor_tensor(
                        out=xtc, in0=xt, scalar=1.0, in1=xb_b,
                        op0=ALU.mult, op1=ALU.subtract)
                    for h in range(nhalf):
                        psy = psY.tile([128, half], f32)
                        for kc in range(KC):
                            nc.tensor.matmul(
                                psy, xtc[:, kc, :],
                                wt[:, kc, h * half:(h + 1) * half],
                                start=(kc == 0), stop=(kc == KC - 1))
                        ysq = ysqp.tile([128, half], fp16, tag="ysq")
                        nc.scalar.square(ysq, psy)
                        nc.tensor.matmul(
                            pvar[:, h, :], e_all[:, c, :], ysq,
                            start=(c == 0), stop=(c == nchunk - 1))
                        # zp = y * prior (fp16) -- frees psy
                        nc.vector.scalar_tensor_tensor(
                            out=zp16[:, c, h * half:(h + 1) * half],
                            in0=psy, scalar=1.0,
                            in1=prior_t[:, h * half:(h + 1) * half],
                            op0=ALU.mult, op1=ALU.mult)

            # ---- stats: s = rsqrt(var + eps), one row per chunk ----------
            with tc.tile_pool(name="statp", bufs=1) as statp:
                s_all16 = statp.tile([nchunk, F], fp16)
                if use_rsqrt:
                    nc.scalar.activation(
                        out=s_all16, in_=pvar.rearrange("p a b -> p (a b)"),
                        func=ACTF.Rsqrt, bias=eps_t, scale=1.0 / VBS)
                else:
                    std_all = statp.tile([nchunk, F], f32)
                    nc.scalar.activation(
                        out=std_all, in_=pvar.rearrange("p a b -> p (a b)"),
                        func=ACTF.Sqrt, bias=eps_t, scale=1.0 / VBS)
                    nc.vector.reciprocal(out=s_all16, in_=std_all)
                nc.sync.dma_start(s16_d[:, :], s_all16)
            psvar_pool.__exit__(None, None, None)

            # ---- phase C: z -> compact -> Newton -> out ------------------
            nc.gpsimd.load_library(libcfg.local_scatter)
            zbig = ctx.enter_context(tc.tile_pool(name="zbig", bufs=2))
            cmp_p = ctx.enter_context(tc.tile_pool(name="cmp", bufs=2))
            cmpi = ctx.enter_context(tc.tile_pool(name="cmpi", bufs=2))
            sbp = ctx.enter_context(tc.tile_pool(name="sbp", bufs=2))
            zcp = ctx.enter_context(tc.tile_pool(name="zcp", bufs=4))
            gsm = ctx.enter_context(tc.tile_pool(name="gsm", bufs=4))
            outp = ctx.enter_context(tc.tile_pool(name="outp", bufs=2))

            for g in range(nchunk // group):
                zss = []
                zcs = []
                zns = []
                for ci in range(group):
                    c = g * group + ci
                    # inv-std row of this chunk, broadcast to all partitions
                    s_sb = sbp.tile([128, F], fp16, tag="s_sb")
                    nc.sync.dma_start(
                        s_sb, bass.AP(tensor=s16_d, offset=c * F,
                                      ap=[[0, 128], [1, F]]))
                    zs = zbig.tile([128, F], fp16, tag="zs_%d" % ci)
                    nc.vector.tensor_mul(zs, zp16[:, c, :], s_sb)
                    zss.append(zs)

                # compact each chunk's candidates (z > thresh); slot 0 is
                # the shared trash slot (rank 0) for non-candidates, whose
                # values are all <= thresh and so never counted by Newton.
                for ci in range(group):
                    mask = cmp_p.tile([128, F], fp16, tag="mask")
                    nc.vector.tensor_scalar(
                        out=mask, in0=zss[ci], scalar1=float(thresh),
                        scalar2=None, op0=ALU.is_gt)
                    csum = cmp_p.tile([128, F], fp16, tag="csum")
                    nc.vector.tensor_tensor_scan(
                        out=csum, data0=mask, data1=mask, initial=0.0,
                        op0=ALU.add, op1=ALU.max)
                    idxt = cmpi.tile([128, F], i16, tag="idx")
                    nc.vector.tensor_mul(idxt, csum, mask)
                    zc = zcp.tile([128, cap], fp16, tag="zc_%d" % ci)
                    nc.gpsimd.local_scatter(
                        out_ap=zc, data_ap=zss[ci],
                        idxs_ap=idxt, channels=128, num_elems=cap,
                        num_idxs=F)
                    zn = zcp.tile([128, cap], fp16, tag="zn_%d" % ci)
                    nc.vector.tensor_scalar(
                        out=zn, in0=zc, scalar1=-1.0,
                        scalar2=None, op0=ALU.mult)
                    zcs.append(zc)
                    zns.append(zn)

                # Newton iterations on the compacted values (batched
                # smalls). K is counted on negated values so only negtau
                # needs updating each iteration.
                negtau = gsm.tile([128, group], f32, tag="negtau")
                nc.vector.memset(negtau, -thresh)
                for it in range(n_iters):
                    racc = gsm.tile([128, group], f32, tag="racc")
                    kacc = gsm.tile([128, group], f32, tag="kacc")
                    for ci in range(group):
                        rs = zcp.tile([128, cap], fp16, tag="rs")
                        ks = zcp.tile([128, cap], fp16, tag="ks")
                        nc.scalar.activation(
                            out=rs, in_=zcs[ci], func=ACTF.Relu,
                            bias=negtau[:, ci:ci + 1],
                            accum_out=racc[:, ci:ci + 1])
                        # count(z > tau) == count(-z < -tau)
                        nc.vector.tensor_scalar(
                            out=ks, in0=zns[ci],
                            scalar1=negtau[:, ci:ci + 1], scalar2=None,
                            op0=ALU.is_lt, op1=ALU.add,
                            accum_out=kacc[:, ci:ci + 1])
                    kinv = gsm.tile([128, group], f32, tag="kinv")
                    nc.vector.reciprocal(out=kinv, in_=kacc)
                    delta = gsm.tile([128, group], f32, tag="delta")
                    nc.vector.scalar_tensor_tensor(
                        out=delta, in0=racc, scalar=-1.0, in1=kinv,
                        op0=ALU.add, op1=ALU.mult)
                    negtau2 = gsm.tile([128, group], f32, tag="negtau")
                    nc.vector.scalar_tensor_tensor(
                        out=negtau2, in0=negtau, scalar=1.0, in1=delta,
                        op0=ALU.mult, op1=ALU.subtract)
                    negtau = negtau2

                # final: out = relu(z - tau)
                for ci in range(group):
                    c = g * group + ci
                    out_t = outp.tile([128, F], fp16, tag="out_t")
                    nc.scalar.activation(
                        out=out_t, in_=zss[ci], func=ACTF.Relu,
                        bias=negtau[:, ci:ci + 1])
                    nc.sync.dma_start(o_d[c * VBS:(c + 1) * VBS, :], out_t)

    nc.compile()
    return nc


_cache = {}


def _get_nc(key, **kw):
    if key not in _cache:
        _cache[key] = build(**kw)
    return _cache[key]


def _run(x, prior_scale, W, gamma, beta, trace=False, **build_kw):
    x16 = np.ascontiguousarray(x, dtype=np.float16)
    p16 = np.ascontiguousarray(prior_scale, dtype=np.float16)
    W16 = np.ascontiguousarray(W, dtype=np.float16)
    gamma = np.asarray(gamma, dtype=np.float32)
    beta = np.asarray(beta, dtype=np.float32)

    nc = _get_nc(("v2", tuple(sorted(build_kw.items()))), **build_kw)

    in_maps = []
    for c in range(N_CORES):
        m = {"x": x16[c * BL:(c + 1) * BL],
             "prior": p16[c * BL:(c + 1) * BL],
             "w": W16}
        in_maps.append(m)

    res = run_bass_kernel_spmd(nc, in_maps, core_ids=list(range(N_CORES)),
                               trace=trace)
    out = np.concatenate(
        [res.results[c]["out"] for c in range(N_CORES)], axis=0)
    out = out.astype(np.float32)
    if not np.all(gamma == 1.0) or not np.all(beta == 0.0):
        raise NotImplementedError("kernel assumes gamma=1, beta=0")
    return out, res


def kernel(x, prior_scale, W, gamma, beta):
    out, _ = _run(x, prior_scale, W, gamma, beta)
    return out


# revision 6
# speedup vs baseline: 1.3245x; 1.3245x over previous
"""AttentiveTransformer (Linear -> ghost BatchNorm -> sparsemax) on 8 TRN2 cores.

Data-parallel over the batch: each core gets 2048 rows (16 ghost-BN chunks of
128 rows). Single matmul pass: y = (x - mean_chunk) @ W.T is computed once per
chunk; zp = y * prior is stored (fp16) while y's squares accumulate chunk
variances in PSUM via one-hot matmuls. Stats (invstd via Rsqrt) are computed
batched for all 16 chunks, then z = zp * invstd. The sparsemax threshold tau
(sum_j relu(z_j - tau) = 1) is found by Newton iteration from tau0 = THRESH,
exact for this piecewise-linear equation. Candidates (z > THRESH) are
compacted to `cap` slots (mask -> scan -> gpsimd local_scatter, slot 0 is a
shared trash slot for non-candidates) and iterations run on compacted values.

x/W/prior are downcast to fp16 on the host; x and W are transposed on the way
into SBUF by the DMA xbar (no PE transposes). Output is written fp16 and
upcast on the host.
"""
import numpy as np
from contextlib import ExitStack

import concourse.bass as bass
import concourse.bacc as bacc
import concourse.tile as tile
import concourse.mybir as mybir
import concourse.library_config as libcfg
from concourse.bass_utils import run_bass_kernel_spmd

N_CORES = 8
B, NA, F = 16384, 512, 2048
BL = B // N_CORES        # rows per core
VBS = 128                # ghost-BN virtual batch
KC = NA // 128           # k-chunks of 128
EPS = 1e-5

f32 = mybir.dt.float32
fp16 = mybir.dt.float16
i16 = mybir.dt.int16
ALU = mybir.AluOpType
ACTF = mybir.ActivationFunctionType


def build(nchunk=BL // VBS, n_iters=7, cap=72, group=4, thresh=1.45,
          half=1024, use_rsqrt=False):
    nc = bacc.Bacc("TRN2", target_bir_lowering=False)
    nhalf = F // half

    Bloc = nchunk * VBS
    x_d = nc.dram_tensor("x", [Bloc, NA], fp16, kind="ExternalInput")
    p_d = nc.dram_tensor("prior", [Bloc, F], fp16, kind="ExternalInput")
    w_d = nc.dram_tensor("w", [F, NA], fp16, kind="ExternalInput")
    o_d = nc.dram_tensor("out", [Bloc, F], fp16, kind="ExternalOutput")
    s16_d = nc.dram_tensor("s16scratch", [nchunk, F], fp16)

    with tile.TileContext(nc) as tc:
        with ExitStack() as ctx:
            ctx.enter_context(nc.allow_low_precision(
                reason="fp16 operands; validated against reference"))
            const = ctx.enter_context(tc.tile_pool(name="const", bufs=1))
            persist = ctx.enter_context(tc.tile_pool(name="persist", bufs=1))
            xp = ctx.enter_context(tc.tile_pool(name="xp", bufs=3))
            priorp = ctx.enter_context(tc.tile_pool(name="priorp", bufs=3))
            small = ctx.enter_context(tc.tile_pool(name="small", bufs=6))
            ysqp = ctx.enter_context(tc.tile_pool(name="ysqp", bufs=3))

            ident = const.tile([128, 128], fp16)
            nc.gpsimd.memset(ident, 0.0)
            nc.gpsimd.affine_select(
                out=ident, in_=ident, compare_op=ALU.not_equal, fill=1.0,
                base=0, pattern=[[-1, 128]], channel_multiplier=1)

            # one-hot columns: e_all[p, c, j] = (c == j)
            e_all = const.tile([128, nchunk, nchunk], fp16)
            nc.gpsimd.memset(e_all, 0.0)
            nc.gpsimd.affine_select(
                out=e_all, in_=e_all, compare_op=ALU.not_equal, fill=1.0,
                base=0, pattern=[[1, nchunk], [-1, nchunk]],
                channel_multiplier=0)

            eps_t = const.tile([nchunk, 1], f32)
            nc.vector.memset(eps_t, EPS)

            # ---- W load + PE transpose: wt[:, kc, f] = W[f, 128*kc+p] ----
            wt = persist.tile([128, KC, F], fp16)
            with tc.tile_pool(name="wtp", bufs=2, space="PSUM") as wtp:
                for ft in range(F // 128):
                    wld = xp.tile([128, NA], fp16, tag="wld")
                    nc.sync.dma_start(wld, w_d[ft * 128:(ft + 1) * 128, :])
                    pst = wtp.tile([128, KC, 128], fp16)
                    for kc in range(KC):
                        nc.tensor.transpose(
                            pst[:, kc, :], wld[:, kc * 128:(kc + 1) * 128],
                            ident)
                    nc.scalar.copy(out=wt[:, :, ft * 128:(ft + 1) * 128],
                                   in_=pst)

            # ---- phase A: per chunk: xT -> center -> matmul -> ysq/zp -----
            zp16 = persist.tile([128, nchunk, F], fp16)
            psvar_pool = tc.tile_pool(name="psvar", bufs=1, space="PSUM")
            psvar = psvar_pool.__enter__()
            pvar = psvar.tile([nchunk, nhalf, half], f32)
            with tc.tile_pool(name="psY", bufs=2, space="PSUM") as psY, \
                 tc.tile_pool(name="psX", bufs=2, space="PSUM") as psX:
                for c in range(nchunk):
                    xld = xp.tile([128, NA], fp16, tag="xld")
                    nc.sync.dma_start(xld, x_d[c * VBS:(c + 1) * VBS, :])
                    xt = psX.tile([128, KC, 128], fp16)
                    for kc in range(KC):
                        nc.tensor.transpose(
                            xt[:, kc, :], xld[:, kc * 128:(kc + 1) * 128],
                            ident)
                    prior_t = priorp.tile([128, F], fp16, tag="prior")
                    nc.sync.dma_start(prior_t, p_d[c * VBS:(c + 1) * VBS, :])
                    xsum = small.tile([128, KC], fp16, tag="xsum")
                    nc.vector.tensor_reduce(
                        out=xsum, in_=xt, axis=mybir.AxisListType.X,
                        op=ALU.add)
                    xbar = small.tile([128, KC], fp16, tag="xbar")
                    nc.vector.tensor_scalar(
                        out=xbar, in0=xsum, scalar1=1.0 / VBS, scalar2=None,
                        op0=ALU.mult)
                    xtc = xp.tile([128, KC, 128], fp16, tag="xtc")
                    xb = xbar[:, :]
                    xb_b = bass.AP(tensor=xb.tensor, offset=xb.offset,
                                   ap=list(xb.ap) + [[0, 128]])
                    nc.vector.scalar_tensor_tensor(
                        out=xtc, in0=xt, scalar=1.0, in1=xb_b,
                        op0=ALU.mult, op1=ALU.subtract)
                    for h in range(nhalf):
                        psy = psY.tile([128, half], f32)
                        for kc in range(KC):
                            nc.tensor.matmul(
                                psy, xtc[:, kc, :],
                                wt[:, kc, h * half:(h + 1) * half],
                                start=(kc == 0), stop=(kc == KC - 1))
                        ysq = ysqp.tile([128, half], fp16, tag="ysq")
                        nc.scalar.square(ysq, psy)
                        nc.tensor.matmul(
                            pvar[:, h, :], e_all[:, c, :], ysq,
                            start=(c == 0), stop=(c == nchunk - 1))
                        # zp = y * prior (fp16) -- frees psy
                        nc.vector.scalar_tensor_tensor(
                            out=zp16[:, c, h * half:(h + 1) * half],
                            in0=psy, scalar=1.0,
                            in1=prior_t[:, h * half:(h + 1) * half],
                            op0=ALU.mult, op1=ALU.mult)

            # ---- stats: s = rsqrt(var + eps), one row per chunk ----------
            with tc.tile_pool(name="statp", bufs=1) as statp:
                s_all16 = statp.tile([nchunk, F], fp16)
                if use_rsqrt:
                    nc.scalar.activation(
                        out=s_all16, in_=pvar.rearrange("p a b -> p (a b)"),
                        func=ACTF.Rsqrt, bias=eps_t, scale=1.0 / VBS)
                else:
                    std_all = statp.tile([nchunk, F], f32)
                    nc.scalar.activation(
                        out=std_all, in_=pvar.rearrange("p a b -> p (a b)"),
                        func=ACTF.Sqrt, bias=eps_t, scale=1.0 / VBS)
                    nc.vector.reciprocal(out=s_all16, in_=std_all)
                nc.sync.dma_start(s16_d[:, :], s_all16)
            psvar_pool.__exit__(None, None, None)

            # ---- phase C: z -> compact -> Newton -> out ------------------
            nc.gpsimd.load_library(libcfg.local_scatter)
            zbig = ctx.enter_context(tc.tile_pool(name="zbig", bufs=2))
            cmp_p = ctx.enter_context(tc.tile_pool(name="cmp", bufs=2))
            cmpi = ctx.enter_context(tc.tile_pool(name="cmpi", bufs=2))
            sbp = ctx.enter_context(tc.tile_pool(name="sbp", bufs=2))
            zcp = ctx.enter_context(tc.tile_pool(name="zcp", bufs=4))
            gsm = ctx.enter_context(tc.tile_pool(name="gsm", bufs=4))
            outp = ctx.enter_context(tc.tile_pool(name="outp", bufs=2))

            for g in range(nchunk // group):
                zss = []
                zcs = []
                zns = []
                for ci in range(group):
                    c = g * group + ci
                    # inv-std row of this chunk, broadcast to all partitions
                    s_sb = sbp.tile([128, F], fp16, tag="s_sb")
                    nc.sync.dma_start(
                        s_sb, bass.AP(tensor=s16_d, offset=c * F,
                                      ap=[[0, 128], [1, F]]))
                    zs = zbig.tile([128, F], fp16, tag="zs_%d" % ci)
                    nc.vector.tensor_mul(zs, zp16[:, c, :], s_sb)
                    zss.append(zs)

                # compact each chunk's candidates (z > thresh); slot 0 is
                # the shared trash slot (rank 0) for non-candidates, whose
                # values are all <= thresh and so never counted by Newton.
                for ci in range(group):
                    mask = cmp_p.tile([128, F], fp16, tag="mask")
                    nc.vector.tensor_scalar(
                        out=mask, in0=zss[ci], scalar1=float(thresh),
                        scalar2=None, op0=ALU.is_gt)
                    csum = cmp_p.tile([128, F], fp16, tag="csum")
                    nc.vector.tensor_tensor_scan(
                        out=csum, data0=mask, data1=mask, initial=0.0,
                        op0=ALU.add, op1=ALU.max)
                    idxt = cmpi.tile([128, F], i16, tag="idx")
                    nc.vector.tensor_mul(idxt, csum, mask)
                    zc = zcp.tile([128, cap], fp16, tag="zc_%d" % ci)
                    nc.gpsimd.local_scatter(
                        out_ap=zc, data_ap=zss[ci],
                        idxs_ap=idxt, channels=128, num_elems=cap,
                        num_idxs=F)
                    zn = zcp.tile([128, cap], fp16, tag="zn_%d" % ci)
                    nc.vector.tensor_scalar(
                        out=zn, in0=zc, scalar1=-1.0,
                        scalar2=None, op0=ALU.mult)
                    zcs.append(zc)
                    zns.append(zn)

                # Newton iterations on the compacted values (batched
                # smalls). K is counted on negated values so only negtau
                # needs updating each iteration.
                negtau = gsm.tile([128, group], f32, tag="negtau")
                nc.vector.memset(negtau, -thresh)
                for it in range(n_iters):
                    racc = gsm.tile([128, group], f32, tag="racc")
                    kacc = gsm.tile([128, group], f32, tag="kacc")
                    for ci in range(group):
                        rs = zcp.tile([128, cap], fp16, tag="rs")
                        ks = zcp.tile([128, cap], fp16, tag="ks")
                        nc.scalar.activation(
                            out=rs, in_=zcs[ci], func=ACTF.Relu,
                            bias=negtau[:, ci:ci + 1],
                            accum_out=racc[:, ci:ci + 1])
                        # count(z > tau) == count(-z < -tau)
                        nc.vector.tensor_scalar(
                            out=ks, in0=zns[ci],
                            scalar1=negtau[:, ci:ci + 1], scalar2=None,
                            op0=ALU.is_lt, op1=ALU.add,
                            accum_out=kacc[:, ci:ci + 1])
                    kinv = gsm.tile([128, group], f32, tag="kinv")
                    nc.vector.reciprocal(out=kinv, in_=kacc)
                    delta = gsm.tile([128, group], f32, tag="delta")
                    nc.vector.scalar_tensor_tensor(
                        out=delta, in0=racc, scalar=-1.0, in1=kinv,
                        op0=ALU.add, op1=ALU.mult)
                    negtau2 = gsm.tile([128, group], f32, tag="negtau")
                    nc.vector.scalar_tensor_tensor(
                        out=negtau2, in0=negtau, scalar=1.0, in1=delta,
                        op0=ALU.mult, op1=ALU.subtract)
                    negtau = negtau2

                # final: out = relu(z - tau)
                for ci in range(group):
                    c = g * group + ci
                    out_t = outp.tile([128, F], fp16, tag="out_t")
                    nc.scalar.activation(
                        out=out_t, in_=zss[ci], func=ACTF.Relu,
                        bias=negtau[:, ci:ci + 1])
                    nc.sync.dma_start(o_d[c * VBS:(c + 1) * VBS, :], out_t)

    nc.compile()
    return nc


_cache = {}


def _get_nc(key, **kw):
    if key not in _cache:
        _cache[key] = build(**kw)
    return _cache[key]


def _run(x, prior_scale, W, gamma, beta, trace=False, **build_kw):
    x16 = np.ascontiguousarray(x, dtype=np.float16)
    p16 = np.ascontiguousarray(prior_scale, dtype=np.float16)
    W16 = np.ascontiguousarray(W, dtype=np.float16)
    gamma = np.asarray(gamma, dtype=np.float32)
    beta = np.asarray(beta, dtype=np.float32)

    nc = _get_nc(("v2", tuple(sorted(build_kw.items()))), **build_kw)

    in_maps = []
    for c in range(N_CORES):
        m = {"x": x16[c * BL:(c + 1) * BL],
             "prior": p16[c * BL:(c + 1) * BL],
             "w": W16}
        in_maps.append(m)

    res = run_bass_kernel_spmd(nc, in_maps, core_ids=list(range(N_CORES)),
                               trace=trace)
    out = np.concatenate(
        [res.results[c]["out"] for c in range(N_CORES)], axis=0)
    out = out.astype(np.float32)
    if not np.all(gamma == 1.0) or not np.all(beta == 0.0):
        raise NotImplementedError("kernel assumes gamma=1, beta=0")
    return out, res


def kernel(x, prior_scale, W, gamma, beta):
    out, _ = _run(x, prior_scale, W, gamma, beta)
    return out


# revision 15
# speedup vs baseline: 1.4009x; 1.0577x over previous
"""AttentiveTransformer (Linear -> ghost BatchNorm -> sparsemax) on 8 TRN2 cores.

Data-parallel over the batch: each core gets 2048 rows (16 ghost-BN chunks of
128 rows). Single matmul pass: y = (x - mean_chunk) @ W.T is computed once per
chunk; zp = y * prior is stored (fp16) while y's squares accumulate chunk
variances in PSUM via one-hot matmuls. Stats (invstd via Rsqrt) are computed
batched for all 16 chunks, then z = zp * invstd. The sparsemax threshold tau
(sum_j relu(z_j - tau) = 1) is found by Newton iteration from tau0 = THRESH,
exact for this piecewise-linear equation. Candidates (z > THRESH) are
compacted to `cap` slots (mask -> scan -> gpsimd local_scatter, slot 0 is a
shared trash slot for non-candidates) and iterations run on compacted values.

x/W/prior are downcast to fp16 on the host; x and W are transposed on the way
into SBUF by the DMA xbar (no PE transposes). Output is written fp16 and
upcast on the host.
"""
import numpy as np
from contextlib import ExitStack

import concourse.bass as bass
import concourse.bacc as bacc
import concourse.tile as tile
import concourse.mybir as mybir
import concourse.library_config as libcfg
from concourse.bass_utils import run_bass_kernel_spmd

N_CORES = 8
B, NA, F = 16384, 512, 2048
BL = B // N_CORES        # rows per core
VBS = 128                # ghost-BN virtual batch
KC = NA // 128           # k-chunks of 128
EPS = 1e-5

f32 = mybir.dt.float32
fp16 = mybir.dt.float16
i16 = mybir.dt.int16
ALU = mybir.AluOpType
ACTF = mybir.ActivationFunctionType


def build(nchunk=BL // VBS, n_iters=5, group=8, thresh=1.45,
          half=512, use_rsqrt=False):
    nc = bacc.Bacc("TRN2", target_bir_lowering=False)
    nhalf = F // half

    Bloc = nchunk * VBS
    x_d = nc.dram_tensor("x", [Bloc, NA], fp16, kind="ExternalInput")
    p_d = nc.dram_tensor("prior", [Bloc, F], fp16, kind="ExternalInput")
    w_d = nc.dram_tensor("w", [F, NA], fp16, kind="ExternalInput")
    o_d = nc.dram_tensor("out", [Bloc, F], fp16, kind="ExternalOutput")
    s16_d = nc.dram_tensor("s16scratch", [nchunk, F], fp16)

    with tile.TileContext(nc) as tc:
        with ExitStack() as ctx:
            ctx.enter_context(nc.allow_low_precision(
                reason="fp16 operands; validated against reference"))
            const = ctx.enter_context(tc.tile_pool(name="const", bufs=1))
            persist = ctx.enter_context(tc.tile_pool(name="persist", bufs=1))
            xp = ctx.enter_context(tc.tile_pool(name="xp", bufs=3))
            priorp = ctx.enter_context(tc.tile_pool(name="priorp", bufs=3))
            small = ctx.enter_context(tc.tile_pool(name="small", bufs=6))
            ysqp = ctx.enter_context(tc.tile_pool(name="ysqp", bufs=3))

            ident = const.tile([128, 128], fp16)
            nc.gpsimd.memset(ident, 0.0)
            nc.gpsimd.affine_select(
                out=ident, in_=ident, compare_op=ALU.not_equal, fill=1.0,
                base=0, pattern=[[-1, 128]], channel_multiplier=1)

            # one-hot columns: e_all[p, c, j] = (c == j)
            e_all = const.tile([128, nchunk, nchunk], fp16)
            nc.gpsimd.memset(e_all, 0.0)
            nc.gpsimd.affine_select(
                out=e_all, in_=e_all, compare_op=ALU.not_equal, fill=1.0,
                base=0, pattern=[[1, nchunk], [-1, nchunk]],
                channel_multiplier=0)

            eps_t = const.tile([nchunk, 1], f32)
            nc.vector.memset(eps_t, EPS)
            zero_t = const.tile([128, 1], f32)
            nc.vector.memset(zero_t, 0.0)

            # ---- W load + PE transpose: wt[:, kc, f] = W[f, 128*kc+p] ----
            wt = persist.tile([128, KC, F], fp16)
            with tc.tile_pool(name="wtp", bufs=2, space="PSUM") as wtp:
                for ft in range(F // 128):
                    wld = xp.tile([128, NA], fp16, tag="wld")
                    nc.sync.dma_start(wld, w_d[ft * 128:(ft + 1) * 128, :])
                    pst = wtp.tile([128, KC, 128], fp16)
                    for kc in range(KC):
                        nc.tensor.transpose(
                            pst[:, kc, :], wld[:, kc * 128:(kc + 1) * 128],
                            ident)
                    nc.scalar.copy(out=wt[:, :, ft * 128:(ft + 1) * 128],
                                   in_=pst)

            # ---- phase A: per chunk: xT -> center -> matmul -> ysq/zp -----
            zp16 = persist.tile([128, nchunk, F], fp16)
            psvar_pool = tc.tile_pool(name="psvar", bufs=1, space="PSUM")
            psvar = psvar_pool.__enter__()
            pvar = psvar.tile([nchunk, nhalf, half], f32)
            with tc.tile_pool(name="psY", bufs=2, space="PSUM") as psY, \
                 tc.tile_pool(name="psX", bufs=2, space="PSUM") as psX:
                for c in range(nchunk):
                    xld = xp.tile([128, NA], fp16, tag="xld")
                    nc.sync.dma_start(xld, x_d[c * VBS:(c + 1) * VBS, :])
                    xt = psX.tile([128, KC, 128], fp16)
                    for kc in range(KC):
                        nc.tensor.transpose(
                            xt[:, kc, :], xld[:, kc * 128:(kc + 1) * 128],
                            ident)
                    prior_t = priorp.tile([128, F], fp16, tag="prior")
                    nc.sync.dma_start(prior_t, p_d[c * VBS:(c + 1) * VBS, :])
                    xsum = small.tile([128, KC], fp16, tag="xsum")
                    nc.vector.tensor_reduce(
                        out=xsum, in_=xt, axis=mybir.AxisListType.X,
                        op=ALU.add)
                    xbar = small.tile([128, KC], fp16, tag="xbar")
                    nc.vector.tensor_scalar(
                        out=xbar, in0=xsum, scalar1=1.0 / VBS, scalar2=None,
                        op0=ALU.mult)
                    xtc = xp.tile([128, KC, 128], fp16, tag="xtc")
                    xb = xbar[:, :]
                    xb_b = bass.AP(tensor=xb.tensor, offset=xb.offset,
                                   ap=list(xb.ap) + [[0, 128]])
                    nc.vector.scalar_tensor_tensor(
                        out=xtc, in0=xt, scalar=1.0, in1=xb_b,
                        op0=ALU.mult, op1=ALU.subtract)
                    for h in range(nhalf):
                        psy = psY.tile([128, half], f32)
                        for kc in range(KC):
                            nc.tensor.matmul(
                                psy, xtc[:, kc, :],
                                wt[:, kc, h * half:(h + 1) * half],
                                start=(kc == 0), stop=(kc == KC - 1))
                        ysq = ysqp.tile([128, half], fp16, tag="ysq")
                        nc.scalar.square(ysq, psy)
                        nc.tensor.matmul(
                            pvar[:, h, :], e_all[:, c, :], ysq,
                            start=(c == 0), stop=(c == nchunk - 1))
                        # zp = y * prior (fp16) -- frees psy
                        nc.vector.scalar_tensor_tensor(
                            out=zp16[:, c, h * half:(h + 1) * half],
                            in0=psy, scalar=1.0,
                            in1=prior_t[:, h * half:(h + 1) * half],
                            op0=ALU.mult, op1=ALU.mult)

            # ---- stats: s = rsqrt(var + eps), one row per chunk ----------
            with tc.tile_pool(name="statp", bufs=1) as statp:
                s_all16 = statp.tile([nchunk, F], fp16)
                if use_rsqrt:
                    nc.scalar.activation(
                        out=s_all16, in_=pvar.rearrange("p a b -> p (a b)"),
                        func=ACTF.Rsqrt, bias=eps_t, scale=1.0 / VBS)
                else:
                    std_all = statp.tile([nchunk, F], f32)
                    nc.scalar.activation(
                        out=std_all, in_=pvar.rearrange("p a b -> p (a b)"),
                        func=ACTF.Sqrt, bias=eps_t, scale=1.0 / VBS)
                    nc.vector.reciprocal(out=s_all16, in_=std_all)
                nc.sync.dma_start(s16_d[:, :], s_all16)
            psvar_pool.__exit__(None, None, None)

            # ---- phase C: z -> pooled Newton -> exact Newton -> out ------
            # Max-pool z by contiguous halving folds (value-preserving
            # subset), run Newton on the 128-wide pooled array (converges
            # from below since pooled r(t) <= r(t)), then finish with
            # n_exact exact full-width steps. No compaction needed.
            zbig = ctx.enter_context(tc.tile_pool(name="zbig", bufs=1))
            scrp = ctx.enter_context(tc.tile_pool(name="scrp", bufs=2))
            sbp = ctx.enter_context(tc.tile_pool(name="sbp", bufs=2))
            foldp = ctx.enter_context(tc.tile_pool(name="foldp", bufs=2))
            zcp = ctx.enter_context(tc.tile_pool(name="zcp", bufs=2))
            gsm = ctx.enter_context(tc.tile_pool(name="gsm", bufs=4))
            outp = ctx.enter_context(tc.tile_pool(name="outp", bufs=2))
            n_pool = n_iters
            n_exact = 3

            for g in range(nchunk // group):
                zss = []
                zps = []
                for ci in range(group):
                    c = g * group + ci
                    # inv-std row of this chunk, broadcast to all partitions
                    s_sb = sbp.tile([128, F], fp16, tag="s_sb")
                    nc.sync.dma_start(
                        s_sb, bass.AP(tensor=s16_d, offset=c * F,
                                      ap=[[0, 128], [1, F]]))
                    zs = zbig.tile([128, F], fp16, tag="zs_%d" % ci)
                    nc.vector.tensor_mul(zs, zp16[:, c, :], s_sb)
                    zss.append(zs)
                    # fold to 128 by halves (contiguous -> 2x DVE mode)
                    f1 = foldp.tile([128, F // 2], fp16, tag="f1")
                    nc.vector.tensor_tensor(
                        out=f1, in0=zs[:, :F // 2], in1=zs[:, F // 2:],
                        op=ALU.max)
                    f2 = foldp.tile([128, F // 4], fp16, tag="f2")
                    nc.vector.tensor_tensor(
                        out=f2, in0=f1[:, :F // 4], in1=f1[:, F // 4:],
                        op=ALU.max)
                    f3 = foldp.tile([128, F // 8], fp16, tag="f3")
                    nc.vector.tensor_tensor(
                        out=f3, in0=f2[:, :F // 8], in1=f2[:, F // 8:],
                        op=ALU.max)
                    zp_ = zcp.tile([128, F // 16], fp16, tag="zp_%d" % ci)
                    nc.vector.tensor_tensor(
                        out=zp_, in0=f3[:, :F // 16], in1=f3[:, F // 16:],
                        op=ALU.max)
                    zps.append(zp_)

                negtau = gsm.tile([128, group], f32, tag="negtau")
                nc.vector.memset(negtau, -thresh)
                for it in range(n_pool + n_exact):
                    exact = it >= n_pool
                    racc = gsm.tile([128, group], f32, tag="racc")
                    kacc = gsm.tile([128, group], f32, tag="kacc")
                    for ci in range(group):
                        if exact:
                            rs = scrp.tile([128, F], fp16, tag="rs")
                            nc.scalar.activation(
                                out=rs, in_=zss[ci], func=ACTF.Relu,
                                bias=negtau[:, ci:ci + 1],
                                accum_out=racc[:, ci:ci + 1])
                            ks = scrp.tile([128, F], fp16, tag="ks")
                        else:
                            rs = zcp.tile([128, F // 16], fp16, tag="rs")
                            nc.scalar.activation(
                                out=rs, in_=zps[ci], func=ACTF.Relu,
                                bias=negtau[:, ci:ci + 1],
                                accum_out=racc[:, ci:ci + 1])
                            ks = zcp.tile([128, F // 16], fp16, tag="ks")
                        # k = #(relu(z - tau) > 0)
                        nc.vector.tensor_scalar(
                            out=ks, in0=rs, scalar1=zero_t[:, 0:1],
                            scalar2=None, op0=ALU.is_gt, op1=ALU.add,
                            accum_out=kacc[:, ci:ci + 1])
                    kinv = gsm.tile([128, group], f32, tag="kinv")
                    nc.vector.reciprocal(out=kinv, in_=kacc)
                    delta = gsm.tile([128, group], f32, tag="delta")
                    nc.vector.scalar_tensor_tensor(
                        out=delta, in0=racc, scalar=-1.0, in1=kinv,
                        op0=ALU.add, op1=ALU.mult)
                    negtau2 = gsm.tile([128, group], f32, tag="negtau")
                    nc.vector.scalar_tensor_tensor(
                        out=negtau2, in0=negtau, scalar=1.0, in1=delta,
                        op0=ALU.mult, op1=ALU.subtract)
                    negtau = negtau2

                # final: out = relu(z - tau)
                for ci in range(group):
                    c = g * group + ci
                    out_t = outp.tile([128, F], fp16, tag="out_t")
                    nc.scalar.activation(
                        out=out_t, in_=zss[ci], func=ACTF.Relu,
                        bias=negtau[:, ci:ci + 1])
                    nc.sync.dma_start(o_d[c * VBS:(c + 1) * VBS, :], out_t)

    nc.compile()
    return nc


_cache = {}


def _get_nc(key, **kw):
    if key not in _cache:
        _cache[key] = build(**kw)
    return _cache[key]


def _run(x, prior_scale, W, gamma, beta, trace=False, **build_kw):
    x16 = np.ascontiguousarray(x, dtype=np.float16)
    p16 = np.ascontiguousarray(prior_scale, dtype=np.float16)
    W16 = np.ascontiguousarray(W, dtype=np.float16)
    gamma = np.asarray(gamma, dtype=np.float32)
    beta = np.asarray(beta, dtype=np.float32)

    nc = _get_nc(("v2", tuple(sorted(build_kw.items()))), **build_kw)

    in_maps = []
    for c in range(N_CORES):
        m = {"x": x16[c * BL:(c + 1) * BL],
             "prior": p16[c * BL:(c + 1) * BL],
             "w": W16}
        in_maps.append(m)

    res = run_bass_kernel_spmd(nc, in_maps, core_ids=list(range(N_CORES)),
                               trace=trace)
    out = np.concatenate(
        [res.results[c]["out"] for c in range(N_CORES)], axis=0)
    out = out.astype(np.float32)
    if not np.all(gamma == 1.0) or not np.all(beta == 0.0):
        raise NotImplementedError("kernel assumes gamma=1, beta=0")
    return out, res


def kernel(x, prior_scale, W, gamma, beta):
    out, _ = _run(x, prior_scale, W, gamma, beta)
    return out
